# revision 17
# baseline (speedup 1.0000x reference)
"""Trainium2 Bass kernel for the nn_DNF problem (8-core SPMD, batch-sharded).

Math (per batch b, permutation p=(a,bo) of 24 objects taken 2, rule r in 0..2,
conjunct c in 0..15, NUM_IN = 32 + 2*64 + 2*64 = 288 features):

  ak    = softmax(and_kernel / T, axis=-1)                  # [3,16,288,3]
  eval  = x*ak0 + (1-x)*ak1 + ak2 = x*d + e   (d = ak0-ak1, e = ak1+ak2)
        = e * (1 + x*f),  f = d/e  (= (ak0-ak1)/(ak1+ak2): softmax-norm free)
  conj[b,p,r,c] = prod_i eval = E[r,c] * prod_i (1 + x_i f_i),
     E[r,c] = prod_i e_i (over all 288 i)
  then probabilistic-OR reductions over p-groups and over c (weighted by
  ok = sigmoid(or_kernel/T)).

Work split across engines (rc = r*16+c in 0..47, DVERC columns on DVE):
 - rc <  DVERC: row-layout on DVE: m = x*f (tensor_tensor bf16 2x), +1
   (tensor_scalar 4x), pairwise-mult halving tree -> per-permutation product.
 - rc >= DVERC: feature-layout: ScalarE computes Ln(x^T*f_rc + 1) fused
   (scale = per-partition f column, bias=1), TensorE sums over the feature
   partition axis via ones-matmuls into PSUM.
 - The per-object -> per-permutation expansions (p//23 / swap) are linear
   maps: mask-matmuls on TensorE, in log domain, accumulated into the same
   PSUM tile; one Exp recovers conj.
 - Unary/nullary row products run on GPSIMD (otherwise idle).
 - OR-reductions over permutation groups: Ln -> masked matmul -> Exp.
"""

from contextlib import ExitStack

import numpy as np
import ml_dtypes

import concourse.bass as bass
import concourse.bacc as bacc
import concourse.mybir as mybir
import concourse.tile as tile
from concourse.bass_utils import run_bass_kernel_spmd

# ---- static problem config (hardcoded; must match the reference) ----
B, O, V = 16, 24, 2
P0, P1, P2 = 32, 64, 64
R, C = 3, 16
NRC = R * C                      # 48
P = O * (O - 1)                  # 552
NCORES = 8
BPC = B // NCORES                # batches per core = 2
NT = (P + 127) // 128            # 5 row-tiles per batch (4x128 + 40)
TROWS = [128, 128, 128, 128, P - 4 * 128]

DVERC = 16                       # conjunct columns computed on the DVE path
LOGRC = NRC - DVERC              # columns on the ScalarE/TensorE log path

F32 = mybir.dt.float32
BF16 = mybir.dt.bfloat16
AL = mybir.AluOpType
ACT = mybir.ActivationFunctionType
AX = mybir.AxisListType

_BF = ml_dtypes.bfloat16
NEG_BIG = -1.0e30                # stand-in for ln(0); 0*NEG_BIG == 0 (no NaN)


def _swap_perm():
    """p=(a,bo) -> index of (bo,a).  p = a*23 + (bo - (bo>a))."""
    sw = np.empty(P, dtype=np.int64)
    p = 0
    for a in range(O):
        for bo in range(O):
            if bo == a:
                continue
            sw[p] = bo * 23 + (a - (a > bo))
            p += 1
    return sw


_SWAP = _swap_perm()


def _or_masks():
    """lhsT for the OR-stage matmuls: [NT, 128, 25] fp32.
    col g<24 = indicator(p//23 == g); col 24 = ones (nullary sum)."""
    m = np.zeros((NT, 128, 25), dtype=np.float32)
    for t in range(NT):
        for r in range(TROWS[t]):
            p = t * 128 + r
            if p < P:
                m[t, r, 24] = 1.0
                m[t, r, p // 23] = 1.0
    return m


def _expand_masks():
    """lhsT for the per-object -> per-permutation expansion matmuls:
    [BPC, NT, BPC*O, 2, 128] fp32. For batch b only rows b*O..b*O+23 are
    nonzero, so the rhs is always the full [BPC*O, NRC] tile (base
    partition 0). [...,0,:] maps rows by a=p//23, [...,1,:] by bo."""
    m = np.zeros((BPC, NT, BPC * O, 2, 128), dtype=np.float32)
    for b in range(BPC):
        for t in range(NT):
            for r in range(TROWS[t]):
                p = t * 128 + r
                if p < P:
                    a = p // 23
                    j = p % 23
                    bo = j + (j >= a)
                    m[b, t, b * O + a, 0, r] = 1.0
                    m[b, t, b * O + bo, 1, r] = 1.0
    return m


_ORMASKS = _or_masks()
_EXMASKS = _expand_masks()


def _build_module():
    nc = bacc.Bacc("TRN2", target_bir_lowering=False, debug=False)

    # ---- I/O ----
    xbin = nc.dram_tensor("xbin", [BPC, P, 128], BF16, kind="ExternalInput")
    xbinT = nc.dram_tensor("xbinT", [BPC, 128, P], BF16, kind="ExternalInput")
    xun = nc.dram_tensor("xun", [BPC * O, P1], BF16, kind="ExternalInput")
    xnul = nc.dram_tensor("xnul", [BPC, P0], BF16, kind="ExternalInput")
    andk = nc.dram_tensor("andk", [R, C, 288, 3], F32, kind="ExternalInput")
    ork = nc.dram_tensor("ork", [R, C], F32, kind="ExternalInput")
    tempr = nc.dram_tensor("tempr", [1, 1], F32, kind="ExternalInput")
    ormasks = nc.dram_tensor("ormasks", [NT, 128, 25], F32, kind="ExternalInput")
    exmasks = nc.dram_tensor("exmasks", [BPC, NT, BPC * O, 2, 128], F32,
                             kind="ExternalInput")

    out_n = nc.dram_tensor("out_n", [BPC, 1], F32, kind="ExternalOutput")
    out_u = nc.dram_tensor("out_u", [BPC, O], F32, kind="ExternalOutput")
    out_b = nc.dram_tensor("out_b", [BPC, P], F32, kind="ExternalOutput")

    # DRAM scratch
    fbin = nc.dram_tensor("fbin", [1, DVERC * 128], BF16)   # f rows, DVE rc only
    fbinT = nc.dram_tensor("fbinT", [1, 128 * NRC], F32)    # f transposed [i, rc]
    fun = nc.dram_tensor("fun", [1, NRC * 128], BF16)
    fnul = nc.dram_tensor("fnul", [1, NRC * P0], BF16)
    escr = nc.dram_tensor("escr", [1, NRC], F32)
    okscr = nc.dram_tensor("okscr", [1, NRC], F32)
    n2scr = nc.dram_tensor("n2scr", [BPC, NRC], F32)

    with tile.TileContext(nc) as tc, ExitStack() as ctx:
        consts = ctx.enter_context(tc.tile_pool(name="consts", bufs=1))
        params = ctx.enter_context(tc.tile_pool(name="params", bufs=1))
        xpool = ctx.enter_context(tc.tile_pool(name="xpool", bufs=3))
        mpool = ctx.enter_context(tc.tile_pool(name="mpool", bufs=2))
        lpool = ctx.enter_context(tc.tile_pool(name="lpool", bufs=1))
        gpool = ctx.enter_context(tc.tile_pool(name="gpool", bufs=3))
        small = ctx.enter_context(tc.tile_pool(name="small", bufs=2))
        psum = ctx.enter_context(tc.tile_pool(name="psum", bufs=1, space="PSUM"))
        psg = ctx.enter_context(tc.tile_pool(name="psg", bufs=2, space="PSUM"))

        # ============ P0: parameters ======================================
        rT = consts.tile([NRC, 1], F32)
        nc.sync.dma_start(out=rT, in_=tempr.ap()[0:1, :].to_broadcast((NRC, 1)))
        nc.vector.reciprocal(out=rT, in_=rT)

        # ok = sigmoid(or_kernel / T)
        ok = consts.tile([1, NRC], F32)
        nc.sync.dma_start(out=ok, in_=ork.ap().rearrange("r c -> () (r c)"))
        nc.vector.tensor_scalar(ok, ok, rT[0:1], None, AL.mult)
        nc.scalar.activation(out=ok, in_=ok, func=ACT.Sigmoid)
        nc.sync.dma_start(out=okscr.ap(), in_=ok)
        okb2 = consts.tile([128, C], F32)
        nc.sync.dma_start(
            out=okb2, in_=bass.AP(tensor=okscr, offset=2 * C, ap=[[0, 128], [1, C]])
        )
        okb1 = consts.tile([O, C], F32)
        nc.sync.dma_start(
            out=okb1, in_=bass.AP(tensor=okscr, offset=C, ap=[[0, O], [1, C]])
        )

        # params: f = (ak0-ak1)/(ak1+ak2); ln E = sum(ln u - ln s)
        ak = params.tile([NRC, 288, 3], F32)
        nc.sync.dma_start(out=ak, in_=andk.ap().rearrange("r c i t -> (r c) i t"))
        nc.vector.tensor_scalar(ak, ak, rT, None, AL.mult)
        nc.scalar.activation(out=ak, in_=ak, func=ACT.Exp)
        umat = params.tile([NRC, 288], F32)
        nc.vector.tensor_tensor(umat, ak[:, :, 1], ak[:, :, 2], AL.add)
        smat = params.tile([NRC, 288], F32)
        nc.vector.tensor_tensor(smat, umat, ak[:, :, 0], AL.add)
        rumat = params.tile([NRC, 288], F32)
        nc.vector.reciprocal(out=rumat, in_=umat)
        dmat = params.tile([NRC, 288], F32)
        nc.vector.tensor_tensor(dmat, ak[:, :, 0], ak[:, :, 1], AL.subtract)
        fmat = params.tile([NRC, 288], F32)
        nc.vector.tensor_tensor(fmat, dmat, rumat, AL.mult)
        fbf = params.tile([NRC, 288], BF16)
        nc.vector.tensor_copy(fbf, fmat)
        lnu = params.tile([NRC, 288], F32)
        nc.scalar.activation(out=lnu, in_=umat, func=ACT.Ln)
        lns = params.tile([NRC, 288], F32)
        nc.scalar.activation(out=lns, in_=smat, func=ACT.Ln)
        nc.vector.tensor_tensor(lnu, lnu, lns, AL.subtract)
        esum = params.tile([NRC, 1], F32)
        nc.vector.tensor_reduce(out=esum, in_=lnu, axis=AX.X, op=AL.add)
        nc.scalar.activation(out=esum, in_=esum, func=ACT.Exp)
        nc.sync.dma_start(out=escr.ap().rearrange("o n -> (o n) ()"), in_=esum)

        # stage f: DVE rows (bf16), transposed fp32 (bin block), unary+nullary
        nc.sync.dma_start(
            out=bass.AP(tensor=fbin, offset=0, ap=[[128, DVERC], [1, 128]]),
            in_=fbf[0:DVERC, 160:288],
        )
        nc.sync.dma_start(
            out=bass.AP(tensor=fbinT, offset=0, ap=[[1, NRC], [NRC, 128]]),
            in_=fmat[:, 160:288],
        )
        nc.sync.dma_start(
            out=bass.AP(tensor=fun, offset=0, ap=[[128, NRC], [1, 128]]),
            in_=fbf[:, 32:160],
        )
        nc.sync.dma_start(
            out=bass.AP(tensor=fnul, offset=0, ap=[[P0, NRC], [1, P0]]),
            in_=fbf[:, 0:32],
        )

        # replicated / transposed f tiles
        frep_b = params.tile([128, DVERC, 128], BF16)
        for s in range(4):
            nc.sync.dma_start(
                out=frep_b[s * 32:(s + 1) * 32],
                in_=bass.AP(tensor=fbin, offset=0, ap=[[0, 32], [1, DVERC * 128]]),
            )
        frep_u = params.tile([BPC * O, NRC, 2, P1], BF16)
        for s in range(2):
            nc.sync.dma_start(
                out=frep_u[s * 24:(s + 1) * 24],
                in_=bass.AP(tensor=fun, offset=0, ap=[[0, 24], [1, NRC * 128]]),
            )
        frep_n = params.tile([BPC, NRC, P0], BF16)
        nc.sync.dma_start(
            out=frep_n,
            in_=bass.AP(tensor=fnul, offset=0, ap=[[0, BPC], [1, NRC * P0]]),
        )
        fT = params.tile([128, NRC], F32)
        nc.sync.dma_start(
            out=fT, in_=bass.AP(tensor=fbinT, offset=0, ap=[[NRC, 128], [1, NRC]])
        )

        omsk = consts.tile([128, NT, 25], F32)
        nc.sync.dma_start(out=omsk, in_=ormasks.ap().rearrange("t r m -> r t m"))
        exm = consts.tile([BPC * O, BPC, NT, 2, 128], F32)
        nc.sync.dma_start(out=exm, in_=exmasks.ap().rearrange("b t o s r -> o b t s r"))

        # ============ P1: unary + nullary products (GPSIMD) ================
        def prod_tree(eng, t_ap, nfeat, out_ap):
            k = nfeat // 2
            while k >= 2:
                eng.tensor_tensor(
                    t_ap[..., 0:k], t_ap[..., 0:k], t_ap[..., k:2 * k], AL.mult
                )
                k //= 2
            eng.tensor_tensor(out_ap, t_ap[..., 0:1], t_ap[..., 1:2], AL.mult)

        xu = xpool.tile([BPC * O, P1], BF16, tag="xu")
        nc.sync.dma_start(out=xu, in_=xun.ap())
        mun = mpool.tile([BPC * O, NRC, 2, P1], BF16, tag="mun")
        nc.gpsimd.tensor_tensor(
            mun,
            xu.unsqueeze(1).unsqueeze(1).to_broadcast((BPC * O, NRC, 2, P1)),
            frep_u,
            AL.mult,
        )
        nc.gpsimd.tensor_scalar(mun, mun, 1.0, None, AL.add)
        uprod = small.tile([BPC * O, NRC, 2, 1], F32, tag="uprod")
        prod_tree(nc.gpsimd, mun, P1, uprod)

        xn = xpool.tile([BPC, P0], BF16, tag="xn")
        nc.sync.dma_start(out=xn, in_=xnul.ap())
        mnul = mpool.tile([BPC, NRC, P0], BF16, tag="mnul")
        nc.gpsimd.tensor_tensor(
            mnul, xn.unsqueeze(1).to_broadcast((BPC, NRC, P0)), frep_n, AL.mult
        )
        nc.gpsimd.tensor_scalar(mnul, mnul, 1.0, None, AL.add)
        nprod = small.tile([BPC, NRC, 1], F32, tag="nprod")
        prod_tree(nc.gpsimd, mnul, P0, nprod)

        # N' = nullary-prod * E ; broadcast rows
        erow = small.tile([BPC, NRC], F32, tag="erow")
        nc.sync.dma_start(
            out=erow, in_=bass.AP(tensor=escr, offset=0, ap=[[0, BPC], [1, NRC]])
        )
        n2 = small.tile([BPC, NRC], F32, tag="n2")
        nc.gpsimd.tensor_tensor(n2, nprod[:, :, 0], erow, AL.mult)
        nc.sync.dma_start(out=n2scr.ap(), in_=n2)
        n2rep = small.tile([BPC * O, NRC], F32, tag="n2rep")
        for b in range(BPC):
            nc.sync.dma_start(
                out=n2rep[b * O:(b + 1) * O],
                in_=bass.AP(tensor=n2scr, offset=b * NRC, ap=[[0, O], [1, NRC]]),
            )
        u1n = small.tile([BPC * O, NRC], F32, tag="u1n")
        nc.gpsimd.tensor_tensor(u1n, uprod[:, :, 0, 0], n2rep, AL.mult)

        # log-domain expansion sources (clamped: ln(0) -> NEG_BIG, not -inf)
        logu1 = small.tile([BPC * O, NRC], F32, tag="logu1")
        nc.scalar.activation(out=logu1, in_=u1n, func=ACT.Ln)
        nc.vector.tensor_scalar(logu1, logu1, NEG_BIG, None, AL.max)
        logu2 = small.tile([BPC * O, NRC], F32, tag="logu2")
        nc.scalar.activation(out=logu2, in_=uprod[:, :, 1, 0], func=ACT.Ln)
        nc.vector.tensor_scalar(logu2, logu2, NEG_BIG, None, AL.max)

        # ============ P2a: log-path feature sums (ScalarE writes L) ========
        xT = xpool.tile([128, BPC, P], BF16, tag="xT")
        for b in range(BPC):
            nc.sync.dma_start(out=xT[:, b, :], in_=xbinT.ap()[b])
        ones_bf = consts.tile([128, 1], BF16)
        nc.vector.memset(ones_bf, 1.0)

        lt = lpool.tile([128, LOGRC, BPC, P], BF16)
        for rc in range(DVERC, NRC):
            nc.scalar.activation(
                out=lt[:, rc - DVERC, :, :],
                in_=xT,
                func=ACT.Ln,
                bias=1.0,
                scale=fT[:, rc:rc + 1],
            )

        # ============ P2b: per-tile pipeline ===============================
        for b in range(BPC):
            gsum = psg.tile([128, NT, NRC], F32, tag="gsum")
            conj = gpool.tile([128, NT, NRC], F32, tag="conj")
            smu = psum.tile([O, C], F32, tag="smu")
            smn = psum.tile([1, C], F32, tag="smn")
            bbts = []
            for t in range(NT):
                rows = TROWS[t]
                # DVE path for rc < DVERC
                xb = xpool.tile([128, 128], BF16, tag="xb")
                nc.sync.dma_start(out=xb[:rows],
                                  in_=xbin.ap()[b, t * 128:t * 128 + rows, :])
                m = mpool.tile([128, DVERC, 128], BF16, tag="m")
                nc.vector.tensor_tensor(
                    m[:rows],
                    xb[:rows].unsqueeze(1).to_broadcast((rows, DVERC, 128)),
                    frep_b[:rows],
                    AL.mult,
                )
                nc.vector.tensor_scalar(m[:rows], m[:rows], 1.0, None, AL.add)
                bbt = gpool.tile([128, DVERC, 1], F32, tag="bbt")
                prod_tree(nc.vector, m[:rows], 128, bbt[:rows])
                bbts.append(bbt)

                # expansions (log domain) + log-path feature sums -> PSUM
                nc.tensor.matmul(gsum[:rows, t, :], exm[:, b, t, 0, :rows],
                                 logu1, start=True, stop=False)
                nc.tensor.matmul(gsum[:rows, t, :], exm[:, b, t, 1, :rows],
                                 logu2, start=False, stop=False)
                for rc in range(DVERC, NRC):
                    nc.tensor.matmul(
                        gsum[:rows, t, rc:rc + 1],
                        lt[:, rc - DVERC, b, t * 128:t * 128 + rows],
                        ones_bf,
                        start=False, stop=(rc == NRC - 1),
                    )

            # conj = exp(gsum); DVE columns then get * bbt
            nc.scalar.activation(out=conj, in_=gsum, func=ACT.Exp)
            for t in range(NT):
                rows = TROWS[t]
                nc.vector.tensor_tensor(
                    conj[:rows, t, 0:DVERC], conj[:rows, t, 0:DVERC],
                    bbts[t][:rows, :, 0], AL.mult,
                )

                # r=2 (binary) out: 1 - prod_c(1 - conj*ok2)
                v = gpool.tile([128, C], F32, tag="v")
                nc.vector.tensor_tensor(v[:rows], conj[:rows, t, 2 * C:3 * C],
                                        okb2[:rows], AL.mult)
                nc.vector.tensor_scalar(v[:rows], v[:rows], -1.0, 1.0, AL.mult, AL.add)
                rm = gpool.tile([128, 1], F32, tag="rm")
                nc.vector.tensor_reduce(out=rm[:rows], in_=v[:rows], axis=AX.X,
                                        op=AL.mult)
                nc.vector.tensor_scalar(rm[:rows], rm[:rows], -1.0, 1.0,
                                        AL.mult, AL.add)
                nc.sync.dma_start(out=out_b.ap()[b:b + 1, t * 128:t * 128 + rows],
                                  in_=rm[:rows])

                # r=0,1: w = ln(1-conj); masked matmul accumulate
                w = gpool.tile([128, 2 * C], F32, tag="w")
                nc.vector.tensor_scalar(w[:rows], conj[:rows, t, 0:2 * C],
                                        -1.0, 1.0, AL.mult, AL.add)
                nc.scalar.activation(out=w[:rows], in_=w[:rows], func=ACT.Ln)
                nc.tensor.matmul(smu, omsk[:rows, t, 0:O], w[:rows, C:2 * C],
                                 start=(t == 0), stop=(t == NT - 1))
                nc.tensor.matmul(smn, omsk[:rows, t, 24:25], w[:rows, 0:C],
                                 start=(t == 0), stop=(t == NT - 1))

            pexp = small.tile([O, C], F32, tag="pexp")
            nc.scalar.activation(out=pexp, in_=smu, func=ACT.Exp)
            pexpn = small.tile([1, C], F32, tag="pexpn")
            nc.scalar.activation(out=pexpn, in_=smn, func=ACT.Exp)

            # nullary out: 1 - prod_c(1 - (1-exp)*ok0)
            t0 = small.tile([1, C], F32, tag="t0")
            nc.vector.tensor_scalar(t0, pexpn, -1.0, 1.0, AL.mult, AL.add)
            nc.vector.tensor_tensor(t0, t0, ok[0:1, 0:C], AL.mult)
            nc.vector.tensor_scalar(t0, t0, -1.0, 1.0, AL.mult, AL.add)
            r0 = small.tile([1, 1], F32, tag="r0")
            nc.vector.tensor_reduce(out=r0, in_=t0, axis=AX.X, op=AL.mult)
            nc.vector.tensor_scalar(r0, r0, -1.0, 1.0, AL.mult, AL.add)
            nc.sync.dma_start(out=out_n.ap()[b:b + 1, :], in_=r0)

            # unary out rows
            t1 = small.tile([O, C], F32, tag="t1")
            nc.vector.tensor_scalar(t1, pexp, -1.0, 1.0, AL.mult, AL.add)
            nc.vector.tensor_tensor(t1, t1, okb1, AL.mult)
            nc.vector.tensor_scalar(t1, t1, -1.0, 1.0, AL.mult, AL.add)
            r1 = small.tile([O, 1], F32, tag="r1")
            nc.vector.tensor_reduce(out=r1, in_=t1, axis=AX.X, op=AL.mult)
            nc.vector.tensor_scalar(r1, r1, -1.0, 1.0, AL.mult, AL.add)
            nc.sync.dma_start(out=out_u.ap()[b:b + 1, :], in_=r1)

    nc.compile()
    return nc


_MODULE = None
LAST_RESULT = None


def _install_ntff_shim():
    """Provide antenv.axon_hooks (NTFF profile hook) if the image lacks it.
    Only matters when BASS_TRACE=1; no-op otherwise."""
    import sys
    import types
    try:
        import antenv.axon_hooks  # noqa: F401
        return
    except ImportError:
        pass
    hook = None
    try:
        from trn_agent_boot.trn_boot import _ntff_profile_via_ctypes
        hook = _ntff_profile_via_ctypes("/opt/axon/libaxon_pjrt.so")
    except Exception:
        hook = None
    mod = types.ModuleType("antenv.axon_hooks")
    state = {"hook": hook}
    mod.get_axon_ntff_profile_hook = lambda: state["hook"]
    mod.set_axon_ntff_profile_hook = lambda h: state.update(hook=h)
    sys.modules["antenv.axon_hooks"] = mod
    try:
        import antenv
        antenv.axon_hooks = mod
    except Exception:
        pass


def kernel(nullary, unary, binary, and_kernel, or_kernel, temperature):
    global _MODULE, LAST_RESULT
    nullary = np.asarray(nullary, dtype=np.float32)
    unary = np.asarray(unary, dtype=np.float32)
    binary = np.asarray(binary, dtype=np.float32)
    and_kernel = np.ascontiguousarray(np.asarray(and_kernel, dtype=np.float32))
    or_kernel = np.ascontiguousarray(np.asarray(or_kernel, dtype=np.float32))
    temperature = np.asarray(temperature, dtype=np.float32)

    # host prep: per-permutation pair rows (+ transposed copy), shard by batch
    binf = binary.reshape(B, P, P2)                       # p = a*23 + idx(bo|a)
    xbin_full = np.concatenate([binf, binf[:, _SWAP, :]], axis=-1).astype(_BF)
    xbinT_full = np.ascontiguousarray(xbin_full.transpose(0, 2, 1))
    xun_full = unary.astype(_BF)
    xnul_full = nullary.astype(_BF)

    if _MODULE is None:
        _MODULE = _build_module()
    nc = _MODULE

    in_maps = []
    for c in range(NCORES):
        b0 = c * BPC
        in_maps.append({
            "xbin": np.ascontiguousarray(xbin_full[b0:b0 + BPC]),
            "xbinT": np.ascontiguousarray(xbinT_full[b0:b0 + BPC]),
            "xun": np.ascontiguousarray(xun_full[b0:b0 + BPC].reshape(BPC * O, P1)),
            "xnul": np.ascontiguousarray(xnul_full[b0:b0 + BPC]),
            "andk": and_kernel,
            "ork": or_kernel,
            "tempr": temperature.reshape(1, 1).astype(np.float32),
            "ormasks": _ORMASKS,
            "exmasks": _EXMASKS,
        })

    _install_ntff_shim()
    LAST_RESULT = run_bass_kernel_spmd(nc, in_maps, core_ids=list(range(NCORES)))
    res = LAST_RESULT.results

    nullary_out = np.concatenate([r["out_n"] for r in res], axis=0).reshape(B, 1)
    unary_out = np.concatenate([r["out_u"] for r in res], axis=0).reshape(B, O, 1)
    binary_out = np.concatenate([r["out_b"] for r in res], axis=0).reshape(B, O, O - 1, 1)
    return nullary_out, unary_out, binary_out


# revision 18
# speedup vs baseline: 1.8526x; 1.8526x over previous
"""Trainium2 Bass kernel for the nn_DNF problem (8-core SPMD, batch-sharded).

Math (per batch b, permutation p=(a,bo) of 24 objects taken 2, rule r in 0..2,
conjunct c in 0..15, NUM_IN = 32 + 2*64 + 2*64 = 288 features):

  ak    = softmax(and_kernel / T, axis=-1)                  # [3,16,288,3]
  eval  = x*ak0 + (1-x)*ak1 + ak2 = x*d + e   (d = ak0-ak1, e = ak1+ak2)
        = e * (1 + x*f),  f = d/e  (= (ak0-ak1)/(ak1+ak2): softmax-norm free)
  conj[b,p,r,c] = prod_i eval = E[r,c] * prod_i (1 + x_i f_i),
     E[r,c] = prod_i e_i (over all 288 i)
  then probabilistic-OR reductions over p-groups and over c (weighted by
  ok = sigmoid(or_kernel/T)).

Work split across engines (rc = r*16+c in 0..47, DVERC columns on DVE):
 - rc <  DVERC: row-layout on DVE: m = x*f (tensor_tensor bf16 2x), +1
   (tensor_scalar 4x), pairwise-mult halving tree -> per-permutation product.
 - rc >= DVERC: feature-layout: ScalarE computes Ln(x^T*f_rc + 1) fused
   (scale = per-partition f column, bias=1), TensorE sums over the feature
   partition axis via ones-matmuls into PSUM.
 - The per-object -> per-permutation expansions (p//23 / swap) are linear
   maps: mask-matmuls on TensorE, in log domain, accumulated into the same
   PSUM tile; one Exp recovers conj.
 - Unary/nullary row products run on GPSIMD (otherwise idle).
 - OR-reductions over permutation groups: Ln -> masked matmul -> Exp.
"""

from contextlib import ExitStack

import numpy as np
import ml_dtypes

import concourse.bass as bass
import concourse.bacc as bacc
import concourse.mybir as mybir
import concourse.tile as tile
from concourse.bass_utils import run_bass_kernel_spmd

# ---- static problem config (hardcoded; must match the reference) ----
B, O, V = 16, 24, 2
P0, P1, P2 = 32, 64, 64
R, C = 3, 16
NRC = R * C                      # 48
P = O * (O - 1)                  # 552
NCORES = 8
BPC = B // NCORES                # batches per core = 2
NT = (P + 127) // 128            # 5 row-tiles per batch (4x128 + 40)
TROWS = [128, 128, 128, 128, P - 4 * 128]

DVERC = 36                       # conjunct columns computed on the DVE path
LOGRC = NRC - DVERC              # columns on the ScalarE/TensorE log path

F32 = mybir.dt.float32
BF16 = mybir.dt.bfloat16
AL = mybir.AluOpType
ACT = mybir.ActivationFunctionType
AX = mybir.AxisListType

_BF = ml_dtypes.bfloat16
NEG_BIG = -1.0e30                # stand-in for ln(0); 0*NEG_BIG == 0 (no NaN)


def _swap_perm():
    """p=(a,bo) -> index of (bo,a).  p = a*23 + (bo - (bo>a))."""
    sw = np.empty(P, dtype=np.int64)
    p = 0
    for a in range(O):
        for bo in range(O):
            if bo == a:
                continue
            sw[p] = bo * 23 + (a - (a > bo))
            p += 1
    return sw


_SWAP = _swap_perm()


def _or_masks():
    """lhsT for the OR-stage matmuls: [NT, 128, 25] fp32.
    col g<24 = indicator(p//23 == g); col 24 = ones (nullary sum)."""
    m = np.zeros((NT, 128, 25), dtype=np.float32)
    for t in range(NT):
        for r in range(TROWS[t]):
            p = t * 128 + r
            if p < P:
                m[t, r, 24] = 1.0
                m[t, r, p // 23] = 1.0
    return m


def _expand_masks():
    """lhsT for the per-object -> per-permutation expansion matmuls:
    [BPC, NT, BPC*O, 2, 128] fp32. For batch b only rows b*O..b*O+23 are
    nonzero, so the rhs is always the full [BPC*O, NRC] tile (base
    partition 0). [...,0,:] maps rows by a=p//23, [...,1,:] by bo."""
    m = np.zeros((BPC, NT, BPC * O, 2, 128), dtype=np.float32)
    for b in range(BPC):
        for t in range(NT):
            for r in range(TROWS[t]):
                p = t * 128 + r
                if p < P:
                    a = p // 23
                    j = p % 23
                    bo = j + (j >= a)
                    m[b, t, b * O + a, 0, r] = 1.0
                    m[b, t, b * O + bo, 1, r] = 1.0
    return m


_ORMASKS = _or_masks()
_EXMASKS = _expand_masks()


def _build_module():
    nc = bacc.Bacc("TRN2", target_bir_lowering=False, debug=False)

    # ---- I/O ----
    xbin = nc.dram_tensor("xbin", [BPC, P, 128], BF16, kind="ExternalInput")
    xbinT = nc.dram_tensor("xbinT", [BPC, 128, P], BF16, kind="ExternalInput")
    xun = nc.dram_tensor("xun", [BPC * O, P1], BF16, kind="ExternalInput")
    xnul = nc.dram_tensor("xnul", [BPC, P0], BF16, kind="ExternalInput")
    andk = nc.dram_tensor("andk", [R, C, 288, 3], F32, kind="ExternalInput")
    ork = nc.dram_tensor("ork", [R, C], F32, kind="ExternalInput")
    tempr = nc.dram_tensor("tempr", [1, 1], F32, kind="ExternalInput")
    ormasks = nc.dram_tensor("ormasks", [NT, 128, 25], F32, kind="ExternalInput")
    exmasks = nc.dram_tensor("exmasks", [BPC, NT, BPC * O, 2, 128], F32,
                             kind="ExternalInput")

    out_n = nc.dram_tensor("out_n", [BPC, 1], F32, kind="ExternalOutput")
    out_u = nc.dram_tensor("out_u", [BPC, O], F32, kind="ExternalOutput")
    out_b = nc.dram_tensor("out_b", [BPC, P], F32, kind="ExternalOutput")

    # DRAM scratch
    fbin = nc.dram_tensor("fbin", [1, DVERC * 128], BF16)   # f rows, DVE rc only
    fbinT = nc.dram_tensor("fbinT", [1, 128 * NRC], F32)    # f transposed [i, rc]
    fun = nc.dram_tensor("fun", [1, NRC * 128], BF16)
    fnul = nc.dram_tensor("fnul", [1, NRC * P0], BF16)
    escr = nc.dram_tensor("escr", [1, NRC], F32)
    okscr = nc.dram_tensor("okscr", [1, NRC], F32)
    n2scr = nc.dram_tensor("n2scr", [BPC, NRC], F32)

    with tile.TileContext(nc) as tc, ExitStack() as ctx:
        consts = ctx.enter_context(tc.tile_pool(name="consts", bufs=1))
        params = ctx.enter_context(tc.tile_pool(name="params", bufs=1))
        xpool = ctx.enter_context(tc.tile_pool(name="xpool", bufs=3))
        mpool = ctx.enter_context(tc.tile_pool(name="mpool", bufs=2))
        lpool = ctx.enter_context(tc.tile_pool(name="lpool", bufs=1))
        gpool = ctx.enter_context(tc.tile_pool(name="gpool", bufs=3))
        small = ctx.enter_context(tc.tile_pool(name="small", bufs=2))
        psum = ctx.enter_context(tc.tile_pool(name="psum", bufs=1, space="PSUM"))
        psg = ctx.enter_context(tc.tile_pool(name="psg", bufs=2, space="PSUM"))

        # ============ P0: parameters ======================================
        rT = consts.tile([NRC, 1], F32)
        nc.sync.dma_start(out=rT, in_=tempr.ap()[0:1, :].to_broadcast((NRC, 1)))
        nc.vector.reciprocal(out=rT, in_=rT)

        # ok = sigmoid(or_kernel / T)
        ok = consts.tile([1, NRC], F32)
        nc.sync.dma_start(out=ok, in_=ork.ap().rearrange("r c -> () (r c)"))
        nc.vector.tensor_scalar(ok, ok, rT[0:1], None, AL.mult)
        nc.scalar.activation(out=ok, in_=ok, func=ACT.Sigmoid)
        nc.sync.dma_start(out=okscr.ap(), in_=ok)
        okb2 = consts.tile([128, C], F32)
        nc.sync.dma_start(
            out=okb2, in_=bass.AP(tensor=okscr, offset=2 * C, ap=[[0, 128], [1, C]])
        )
        okb1 = consts.tile([O, C], F32)
        nc.sync.dma_start(
            out=okb1, in_=bass.AP(tensor=okscr, offset=C, ap=[[0, O], [1, C]])
        )

        # params: f = (ak0-ak1)/(ak1+ak2); ln E = sum(ln u - ln s)
        ak = params.tile([NRC, 288, 3], F32)
        nc.sync.dma_start(out=ak, in_=andk.ap().rearrange("r c i t -> (r c) i t"))
        nc.vector.tensor_scalar(ak, ak, rT, None, AL.mult)
        nc.scalar.activation(out=ak, in_=ak, func=ACT.Exp)
        umat = params.tile([NRC, 288], F32)
        nc.vector.tensor_tensor(umat, ak[:, :, 1], ak[:, :, 2], AL.add)
        smat = params.tile([NRC, 288], F32)
        nc.vector.tensor_tensor(smat, umat, ak[:, :, 0], AL.add)
        rumat = params.tile([NRC, 288], F32)
        nc.vector.reciprocal(out=rumat, in_=umat)
        dmat = params.tile([NRC, 288], F32)
        nc.vector.tensor_tensor(dmat, ak[:, :, 0], ak[:, :, 1], AL.subtract)
        fmat = params.tile([NRC, 288], F32)
        nc.vector.tensor_tensor(fmat, dmat, rumat, AL.mult)
        fbf = params.tile([NRC, 288], BF16)
        nc.vector.tensor_copy(fbf, fmat)
        lnu = params.tile([NRC, 288], F32)
        nc.scalar.activation(out=lnu, in_=umat, func=ACT.Ln)
        lns = params.tile([NRC, 288], F32)
        nc.scalar.activation(out=lns, in_=smat, func=ACT.Ln)
        nc.vector.tensor_tensor(lnu, lnu, lns, AL.subtract)
        esum = params.tile([NRC, 1], F32)
        nc.vector.tensor_reduce(out=esum, in_=lnu, axis=AX.X, op=AL.add)
        nc.scalar.activation(out=esum, in_=esum, func=ACT.Exp)
        nc.sync.dma_start(out=escr.ap().rearrange("o n -> (o n) ()"), in_=esum)

        # stage f: DVE rows (bf16), transposed fp32 (bin block), unary+nullary
        nc.sync.dma_start(
            out=bass.AP(tensor=fbin, offset=0, ap=[[128, DVERC], [1, 128]]),
            in_=fbf[0:DVERC, 160:288],
        )
        nc.sync.dma_start(
            out=bass.AP(tensor=fbinT, offset=0, ap=[[1, NRC], [NRC, 128]]),
            in_=fmat[:, 160:288],
        )
        nc.sync.dma_start(
            out=bass.AP(tensor=fun, offset=0, ap=[[128, NRC], [1, 128]]),
            in_=fbf[:, 32:160],
        )
        nc.sync.dma_start(
            out=bass.AP(tensor=fnul, offset=0, ap=[[P0, NRC], [1, P0]]),
            in_=fbf[:, 0:32],
        )

        # replicated / transposed f tiles
        frep_b = params.tile([128, DVERC, 128], BF16)
        for s in range(4):
            nc.sync.dma_start(
                out=frep_b[s * 32:(s + 1) * 32],
                in_=bass.AP(tensor=fbin, offset=0, ap=[[0, 32], [1, DVERC * 128]]),
            )
        frep_u = params.tile([BPC * O, NRC, 2, P1], BF16)
        for s in range(2):
            nc.sync.dma_start(
                out=frep_u[s * 24:(s + 1) * 24],
                in_=bass.AP(tensor=fun, offset=0, ap=[[0, 24], [1, NRC * 128]]),
            )
        frep_n = params.tile([BPC, NRC, P0], BF16)
        nc.sync.dma_start(
            out=frep_n,
            in_=bass.AP(tensor=fnul, offset=0, ap=[[0, BPC], [1, NRC * P0]]),
        )
        fT = params.tile([128, NRC], F32)
        nc.sync.dma_start(
            out=fT, in_=bass.AP(tensor=fbinT, offset=0, ap=[[NRC, 128], [1, NRC]])
        )

        omsk = consts.tile([128, NT, 25], F32)
        nc.sync.dma_start(out=omsk, in_=ormasks.ap().rearrange("t r m -> r t m"))
        exm = consts.tile([BPC * O, BPC, NT, 2, 128], F32)
        nc.sync.dma_start(out=exm, in_=exmasks.ap().rearrange("b t o s r -> o b t s r"))

        # ============ P1: unary + nullary products (GPSIMD) ================
        def prod_tree(eng, t_ap, nfeat, out_ap):
            k = nfeat // 2
            while k >= 2:
                eng.tensor_tensor(
                    t_ap[..., 0:k], t_ap[..., 0:k], t_ap[..., k:2 * k], AL.mult
                )
                k //= 2
            eng.tensor_tensor(out_ap, t_ap[..., 0:1], t_ap[..., 1:2], AL.mult)

        xu = xpool.tile([BPC * O, P1], BF16, tag="xu")
        nc.sync.dma_start(out=xu, in_=xun.ap())
        mun = mpool.tile([BPC * O, NRC, 2, P1], BF16, tag="mun")
        nc.vector.tensor_tensor(
            mun,
            xu.unsqueeze(1).unsqueeze(1).to_broadcast((BPC * O, NRC, 2, P1)),
            frep_u,
            AL.mult,
        )
        nc.scalar.activation(out=mun, in_=mun, func=ACT.Copy, bias=1.0)
        uprod = small.tile([BPC * O, NRC, 2, 1], F32, tag="uprod")
        prod_tree(nc.vector, mun, P1, uprod)

        xn = xpool.tile([BPC, P0], BF16, tag="xn")
        nc.sync.dma_start(out=xn, in_=xnul.ap())
        mnul = mpool.tile([BPC, NRC, P0], BF16, tag="mnul")
        nc.vector.tensor_tensor(
            mnul, xn.unsqueeze(1).to_broadcast((BPC, NRC, P0)), frep_n, AL.mult
        )
        nc.scalar.activation(out=mnul, in_=mnul, func=ACT.Copy, bias=1.0)
        nprod = small.tile([BPC, NRC, 1], F32, tag="nprod")
        prod_tree(nc.vector, mnul, P0, nprod)

        # N' = nullary-prod * E ; broadcast rows
        erow = small.tile([BPC, NRC], F32, tag="erow")
        nc.sync.dma_start(
            out=erow, in_=bass.AP(tensor=escr, offset=0, ap=[[0, BPC], [1, NRC]])
        )
        n2 = small.tile([BPC, NRC], F32, tag="n2")
        nc.vector.tensor_tensor(n2, nprod[:, :, 0], erow, AL.mult)
        nc.sync.dma_start(out=n2scr.ap(), in_=n2)
        n2rep = small.tile([BPC * O, NRC], F32, tag="n2rep")
        for b in range(BPC):
            nc.sync.dma_start(
                out=n2rep[b * O:(b + 1) * O],
                in_=bass.AP(tensor=n2scr, offset=b * NRC, ap=[[0, O], [1, NRC]]),
            )
        u1n = small.tile([BPC * O, NRC], F32, tag="u1n")
        nc.vector.tensor_tensor(u1n, uprod[:, :, 0, 0], n2rep, AL.mult)

        # log-domain expansion sources (clamped: ln(0) -> NEG_BIG, not -inf)
        logu1 = small.tile([BPC * O, NRC], F32, tag="logu1")
        nc.scalar.activation(out=logu1, in_=u1n, func=ACT.Ln)
        nc.vector.tensor_scalar(logu1, logu1, NEG_BIG, None, AL.max)
        logu2 = small.tile([BPC * O, NRC], F32, tag="logu2")
        nc.scalar.activation(out=logu2, in_=uprod[:, :, 1, 0], func=ACT.Ln)
        nc.vector.tensor_scalar(logu2, logu2, NEG_BIG, None, AL.max)

        # ============ P2a: log-path feature sums (ScalarE writes L) ========
        xT = xpool.tile([128, BPC, P], BF16, tag="xT")
        for b in range(BPC):
            nc.sync.dma_start(out=xT[:, b, :], in_=xbinT.ap()[b])
        ones_bf = consts.tile([128, 1], BF16)
        nc.vector.memset(ones_bf, 1.0)

        lt = lpool.tile([128, LOGRC, BPC, P], BF16)
        for rc in range(DVERC, NRC):
            nc.scalar.activation(
                out=lt[:, rc - DVERC, :, :],
                in_=xT,
                func=ACT.Ln,
                bias=1.0,
                scale=fT[:, rc:rc + 1],
            )

        # ============ P2b: per-tile pipeline ===============================
        for b in range(BPC):
            gsum = psg.tile([128, NT, NRC], F32, tag="gsum")
            conj = gpool.tile([128, NT, NRC], F32, tag="conj")
            smu = psum.tile([O, C], F32, tag="smu")
            smn = psum.tile([1, C], F32, tag="smn")
            bbts = []
            for t in range(NT):
                rows = TROWS[t]
                # DVE path for rc < DVERC
                xb = xpool.tile([128, 128], BF16, tag="xb")
                nc.sync.dma_start(out=xb[:rows],
                                  in_=xbin.ap()[b, t * 128:t * 128 + rows, :])
                m = mpool.tile([128, DVERC, 128], BF16, tag="m")
                nc.vector.tensor_tensor(
                    m[:rows],
                    xb[:rows].unsqueeze(1).to_broadcast((rows, DVERC, 128)),
                    frep_b[:rows],
                    AL.mult,
                )
                nc.scalar.activation(out=m[:rows], in_=m[:rows], func=ACT.Copy,
                                     bias=1.0)
                bbt = gpool.tile([128, DVERC, 1], F32, tag="bbt")
                k = 64
                while k >= 8:
                    nc.vector.tensor_tensor(m[:rows, :, 0:k], m[:rows, :, 0:k],
                                            m[:rows, :, k:2 * k], AL.mult)
                    k //= 2
                nc.vector.tensor_reduce(out=bbt[:rows], in_=m[:rows, :, 0:8],
                                        axis=AX.X, op=AL.mult)
                bbts.append(bbt)

                # expansions (log domain) + log-path feature sums -> PSUM
                nc.tensor.matmul(gsum[:rows, t, :], exm[:, b, t, 0, :rows],
                                 logu1, start=True, stop=False)
                nc.tensor.matmul(gsum[:rows, t, :], exm[:, b, t, 1, :rows],
                                 logu2, start=False, stop=False)
                for rc in range(DVERC, NRC):
                    nc.tensor.matmul(
                        gsum[:rows, t, rc:rc + 1],
                        lt[:, rc - DVERC, b, t * 128:t * 128 + rows],
                        ones_bf,
                        start=False, stop=(rc == NRC - 1),
                    )

            # conj = exp(gsum); DVE columns then get * bbt
            nc.scalar.activation(out=conj, in_=gsum, func=ACT.Exp)
            for t in range(NT):
                rows = TROWS[t]
                nc.vector.tensor_tensor(
                    conj[:rows, t, 0:DVERC], conj[:rows, t, 0:DVERC],
                    bbts[t][:rows, :, 0], AL.mult,
                )

                # r=2 (binary) out: 1 - prod_c(1 - conj*ok2)
                v = gpool.tile([128, C], F32, tag="v")
                nc.vector.tensor_tensor(v[:rows], conj[:rows, t, 2 * C:3 * C],
                                        okb2[:rows], AL.mult)
                nc.vector.tensor_scalar(v[:rows], v[:rows], -1.0, 1.0, AL.mult, AL.add)
                rm = gpool.tile([128, 1], F32, tag="rm")
                nc.vector.tensor_reduce(out=rm[:rows], in_=v[:rows], axis=AX.X,
                                        op=AL.mult)
                nc.vector.tensor_scalar(rm[:rows], rm[:rows], -1.0, 1.0,
                                        AL.mult, AL.add)
                nc.sync.dma_start(out=out_b.ap()[b:b + 1, t * 128:t * 128 + rows],
                                  in_=rm[:rows])

                # r=0,1: w = ln(1-conj); masked matmul accumulate
                w = gpool.tile([128, 2 * C], F32, tag="w")
                nc.vector.tensor_scalar(w[:rows], conj[:rows, t, 0:2 * C],
                                        -1.0, 1.0, AL.mult, AL.add)
                nc.scalar.activation(out=w[:rows], in_=w[:rows], func=ACT.Ln)
                nc.tensor.matmul(smu, omsk[:rows, t, 0:O], w[:rows, C:2 * C],
                                 start=(t == 0), stop=(t == NT - 1))
                nc.tensor.matmul(smn, omsk[:rows, t, 24:25], w[:rows, 0:C],
                                 start=(t == 0), stop=(t == NT - 1))

            pexp = small.tile([O, C], F32, tag="pexp")
            nc.scalar.activation(out=pexp, in_=smu, func=ACT.Exp)
            pexpn = small.tile([1, C], F32, tag="pexpn")
            nc.scalar.activation(out=pexpn, in_=smn, func=ACT.Exp)

            # nullary out: 1 - prod_c(1 - (1-exp)*ok0)
            t0 = small.tile([1, C], F32, tag="t0")
            nc.vector.tensor_scalar(t0, pexpn, -1.0, 1.0, AL.mult, AL.add)
            nc.vector.tensor_tensor(t0, t0, ok[0:1, 0:C], AL.mult)
            nc.vector.tensor_scalar(t0, t0, -1.0, 1.0, AL.mult, AL.add)
            r0 = small.tile([1, 1], F32, tag="r0")
            nc.vector.tensor_reduce(out=r0, in_=t0, axis=AX.X, op=AL.mult)
            nc.vector.tensor_scalar(r0, r0, -1.0, 1.0, AL.mult, AL.add)
            nc.sync.dma_start(out=out_n.ap()[b:b + 1, :], in_=r0)

            # unary out rows
            t1 = small.tile([O, C], F32, tag="t1")
            nc.vector.tensor_scalar(t1, pexp, -1.0, 1.0, AL.mult, AL.add)
            nc.vector.tensor_tensor(t1, t1, okb1, AL.mult)
            nc.vector.tensor_scalar(t1, t1, -1.0, 1.0, AL.mult, AL.add)
            r1 = small.tile([O, 1], F32, tag="r1")
            nc.vector.tensor_reduce(out=r1, in_=t1, axis=AX.X, op=AL.mult)
            nc.vector.tensor_scalar(r1, r1, -1.0, 1.0, AL.mult, AL.add)
            nc.sync.dma_start(out=out_u.ap()[b:b + 1, :], in_=r1)

    nc.compile()
    return nc


_MODULE = None
LAST_RESULT = None


def _install_ntff_shim():
    """Provide antenv.axon_hooks (NTFF profile hook) if the image lacks it.
    Only matters when BASS_TRACE=1; no-op otherwise."""
    import sys
    import types
    try:
        import antenv.axon_hooks  # noqa: F401
        return
    except ImportError:
        pass
    hook = None
    try:
        from trn_agent_boot.trn_boot import _ntff_profile_via_ctypes
        hook = _ntff_profile_via_ctypes("/opt/axon/libaxon_pjrt.so")
    except Exception:
        hook = None
    mod = types.ModuleType("antenv.axon_hooks")
    state = {"hook": hook}
    mod.get_axon_ntff_profile_hook = lambda: state["hook"]
    mod.set_axon_ntff_profile_hook = lambda h: state.update(hook=h)
    sys.modules["antenv.axon_hooks"] = mod
    try:
        import antenv
        antenv.axon_hooks = mod
    except Exception:
        pass


def kernel(nullary, unary, binary, and_kernel, or_kernel, temperature):
    global _MODULE, LAST_RESULT
    nullary = np.asarray(nullary, dtype=np.float32)
    unary = np.asarray(unary, dtype=np.float32)
    binary = np.asarray(binary, dtype=np.float32)
    and_kernel = np.ascontiguousarray(np.asarray(and_kernel, dtype=np.float32))
    or_kernel = np.ascontiguousarray(np.asarray(or_kernel, dtype=np.float32))
    temperature = np.asarray(temperature, dtype=np.float32)

    # host prep: per-permutation pair rows (+ transposed copy), shard by batch
    binf = binary.reshape(B, P, P2)                       # p = a*23 + idx(bo|a)
    xbin_full = np.concatenate([binf, binf[:, _SWAP, :]], axis=-1).astype(_BF)
    xbinT_full = np.ascontiguousarray(xbin_full.transpose(0, 2, 1))
    xun_full = unary.astype(_BF)
    xnul_full = nullary.astype(_BF)

    if _MODULE is None:
        _MODULE = _build_module()
    nc = _MODULE

    in_maps = []
    for c in range(NCORES):
        b0 = c * BPC
        in_maps.append({
            "xbin": np.ascontiguousarray(xbin_full[b0:b0 + BPC]),
            "xbinT": np.ascontiguousarray(xbinT_full[b0:b0 + BPC]),
            "xun": np.ascontiguousarray(xun_full[b0:b0 + BPC].reshape(BPC * O, P1)),
            "xnul": np.ascontiguousarray(xnul_full[b0:b0 + BPC]),
            "andk": and_kernel,
            "ork": or_kernel,
            "tempr": temperature.reshape(1, 1).astype(np.float32),
            "ormasks": _ORMASKS,
            "exmasks": _EXMASKS,
        })

    _install_ntff_shim()
    LAST_RESULT = run_bass_kernel_spmd(nc, in_maps, core_ids=list(range(NCORES)))
    res = LAST_RESULT.results

    nullary_out = np.concatenate([r["out_n"] for r in res], axis=0).reshape(B, 1)
    unary_out = np.concatenate([r["out_u"] for r in res], axis=0).reshape(B, O, 1)
    binary_out = np.concatenate([r["out_b"] for r in res], axis=0).reshape(B, O, O - 1, 1)
    return nullary_out, unary_out, binary_out


# revision 20
# speedup vs baseline: 1.8910x; 1.0207x over previous
"""Trainium2 Bass kernel for the nn_DNF problem (8-core SPMD, batch-sharded).

Math (per batch b, permutation p=(a,bo) of 24 objects taken 2, rule r in 0..2,
conjunct c in 0..15, NUM_IN = 32 + 2*64 + 2*64 = 288 features):

  ak    = softmax(and_kernel / T, axis=-1)                  # [3,16,288,3]
  eval  = x*ak0 + (1-x)*ak1 + ak2 = x*d + e   (d = ak0-ak1, e = ak1+ak2)
        = e * (1 + x*f),  f = d/e  (= (ak0-ak1)/(ak1+ak2): softmax-norm free)
  conj[b,p,r,c] = prod_i eval = E[r,c] * prod_i (1 + x_i f_i),
     E[r,c] = prod_i e_i (over all 288 i)
  then probabilistic-OR reductions over p-groups and over c (weighted by
  ok = sigmoid(or_kernel/T)).

Work split across engines (rc = r*16+c in 0..47, DVERC columns on DVE):
 - rc <  DVERC: row-layout on DVE: m = x*f (tensor_tensor bf16 2x), +1
   (tensor_scalar 4x), pairwise-mult halving tree -> per-permutation product.
 - rc >= DVERC: feature-layout: ScalarE computes Ln(x^T*f_rc + 1) fused
   (scale = per-partition f column, bias=1), TensorE sums over the feature
   partition axis via ones-matmuls into PSUM.
 - The per-object -> per-permutation expansions (p//23 / swap) are linear
   maps: mask-matmuls on TensorE, in log domain, accumulated into the same
   PSUM tile; one Exp recovers conj.
 - Unary/nullary row products run on GPSIMD (otherwise idle).
 - OR-reductions over permutation groups: Ln -> masked matmul -> Exp.
"""

from contextlib import ExitStack

import numpy as np
import ml_dtypes

import concourse.bass as bass
import concourse.bacc as bacc
import concourse.mybir as mybir
import concourse.tile as tile
from concourse.bass_utils import run_bass_kernel_spmd

# ---- static problem config (hardcoded; must match the reference) ----
B, O, V = 16, 24, 2
P0, P1, P2 = 32, 64, 64
R, C = 3, 16
NRC = R * C                      # 48
P = O * (O - 1)                  # 552
NCORES = 8
BPC = B // NCORES                # batches per core = 2
NT = (P + 127) // 128            # 5 row-tiles per batch (4x128 + 40)
TROWS = [128, 128, 128, 128, P - 4 * 128]

DVERC = 36                       # conjunct columns computed on the DVE path
LOGRC = NRC - DVERC              # columns on the ScalarE/TensorE log path

F32 = mybir.dt.float32
BF16 = mybir.dt.bfloat16
AL = mybir.AluOpType
ACT = mybir.ActivationFunctionType
AX = mybir.AxisListType

_BF = ml_dtypes.bfloat16
NEG_BIG = -1.0e30                # stand-in for ln(0); 0*NEG_BIG == 0 (no NaN)


def _swap_perm():
    """p=(a,bo) -> index of (bo,a).  p = a*23 + (bo - (bo>a))."""
    sw = np.empty(P, dtype=np.int64)
    p = 0
    for a in range(O):
        for bo in range(O):
            if bo == a:
                continue
            sw[p] = bo * 23 + (a - (a > bo))
            p += 1
    return sw


_SWAP = _swap_perm()


def _or_masks():
    """lhsT for the OR-stage matmuls: [NT, 128, 25] fp32.
    col g<24 = indicator(p//23 == g); col 24 = ones (nullary sum)."""
    m = np.zeros((NT, 128, 25), dtype=np.float32)
    for t in range(NT):
        for r in range(TROWS[t]):
            p = t * 128 + r
            if p < P:
                m[t, r, 24] = 1.0
                m[t, r, p // 23] = 1.0
    return m


def _expand_masks():
    """lhsT for the per-object -> per-permutation expansion matmuls:
    [BPC, NT, BPC*O, 2, 128] fp32. For batch b only rows b*O..b*O+23 are
    nonzero, so the rhs is always the full [BPC*O, NRC] tile (base
    partition 0). [...,0,:] maps rows by a=p//23, [...,1,:] by bo."""
    m = np.zeros((BPC, NT, BPC * O, 2, 128), dtype=np.float32)
    for b in range(BPC):
        for t in range(NT):
            for r in range(TROWS[t]):
                p = t * 128 + r
                if p < P:
                    a = p // 23
                    j = p % 23
                    bo = j + (j >= a)
                    m[b, t, b * O + a, 0, r] = 1.0
                    m[b, t, b * O + bo, 1, r] = 1.0
    return m


_ORMASKS = _or_masks()
_EXMASKS = _expand_masks().astype(_BF)


def _build_module():
    nc = bacc.Bacc("TRN2", target_bir_lowering=False, debug=False)

    # ---- I/O ----
    xbin = nc.dram_tensor("xbin", [BPC, P, 128], BF16, kind="ExternalInput")
    xbinT = nc.dram_tensor("xbinT", [BPC, 128, P], BF16, kind="ExternalInput")
    xun = nc.dram_tensor("xun", [BPC * O, P1], BF16, kind="ExternalInput")
    xnul = nc.dram_tensor("xnul", [BPC, P0], BF16, kind="ExternalInput")
    andk = nc.dram_tensor("andk", [R, C, 288, 3], F32, kind="ExternalInput")
    ork = nc.dram_tensor("ork", [R, C], F32, kind="ExternalInput")
    tempr = nc.dram_tensor("tempr", [1, 1], F32, kind="ExternalInput")
    ormasks = nc.dram_tensor("ormasks", [NT, 128, 25], F32, kind="ExternalInput")
    exmasks = nc.dram_tensor("exmasks", [BPC, NT, BPC * O, 2, 128], BF16,
                             kind="ExternalInput")

    out_n = nc.dram_tensor("out_n", [BPC, 1], F32, kind="ExternalOutput")
    out_u = nc.dram_tensor("out_u", [BPC, O], F32, kind="ExternalOutput")
    out_b = nc.dram_tensor("out_b", [BPC, P], F32, kind="ExternalOutput")

    # DRAM scratch
    fbin = nc.dram_tensor("fbin", [1, DVERC * 128], BF16)   # f rows, DVE rc only
    fbinT = nc.dram_tensor("fbinT", [1, 128 * NRC], F32)    # f transposed [i, rc]
    fun = nc.dram_tensor("fun", [1, NRC * 128], BF16)
    fnul = nc.dram_tensor("fnul", [1, NRC * P0], BF16)
    escr = nc.dram_tensor("escr", [1, NRC], F32)
    okscr = nc.dram_tensor("okscr", [1, NRC], F32)
    n2scr = nc.dram_tensor("n2scr", [BPC, NRC], F32)

    with tile.TileContext(nc) as tc, ExitStack() as ctx:
        consts = ctx.enter_context(tc.tile_pool(name="consts", bufs=1))
        params = ctx.enter_context(tc.tile_pool(name="params", bufs=1))
        xpool = ctx.enter_context(tc.tile_pool(name="xpool", bufs=3))
        mpool = ctx.enter_context(tc.tile_pool(name="mpool", bufs=2))
        lpool = ctx.enter_context(tc.tile_pool(name="lpool", bufs=1))
        gpool = ctx.enter_context(tc.tile_pool(name="gpool", bufs=3))
        small = ctx.enter_context(tc.tile_pool(name="small", bufs=2))
        psum = ctx.enter_context(tc.tile_pool(name="psum", bufs=1, space="PSUM"))
        psg = ctx.enter_context(tc.tile_pool(name="psg", bufs=2, space="PSUM"))

        # ============ prefetch: const + data loads (gpsimd DMA queue) =====
        omsk = consts.tile([128, NT, 25], F32)
        nc.gpsimd.dma_start(out=omsk, in_=ormasks.ap().rearrange("t r m -> r t m"))
        exm = consts.tile([BPC * O, BPC, NT, 2, 128], BF16)
        nc.gpsimd.dma_start(out=exm,
                            in_=exmasks.ap().rearrange("b t o s r -> o b t s r"))
        xT = xpool.tile([128, BPC, P], BF16, tag="xT")
        for b in range(BPC):
            nc.gpsimd.dma_start(out=xT[:, b, :], in_=xbinT.ap()[b])
        xball = xpool.tile([128, BPC * NT, 128], BF16, tag="xball")
        for b in range(BPC):
            for t in range(NT):
                rows = TROWS[t]
                nc.gpsimd.dma_start(
                    out=xball[:rows, b * NT + t, :],
                    in_=xbin.ap()[b, t * 128:t * 128 + rows, :])
        xu = xpool.tile([BPC * O, P1], BF16, tag="xu")
        nc.gpsimd.dma_start(out=xu, in_=xun.ap())
        xn = xpool.tile([BPC, P0], BF16, tag="xn")
        nc.gpsimd.dma_start(out=xn, in_=xnul.ap())
        ones_bf = consts.tile([128, 1], BF16)
        nc.vector.memset(ones_bf, 1.0)

        # ============ P0: parameters ======================================
        rT = consts.tile([NRC, 1], F32)
        nc.sync.dma_start(out=rT, in_=tempr.ap()[0:1, :].to_broadcast((NRC, 1)))
        nc.vector.reciprocal(out=rT, in_=rT)

        # ok = sigmoid(or_kernel / T)
        ok = consts.tile([1, NRC], F32)
        nc.sync.dma_start(out=ok, in_=ork.ap().rearrange("r c -> () (r c)"))
        nc.vector.tensor_scalar(ok, ok, rT[0:1], None, AL.mult)
        nc.scalar.activation(out=ok, in_=ok, func=ACT.Sigmoid)
        nc.sync.dma_start(out=okscr.ap(), in_=ok)
        okb2 = consts.tile([128, C], F32)
        nc.sync.dma_start(
            out=okb2, in_=bass.AP(tensor=okscr, offset=2 * C, ap=[[0, 128], [1, C]])
        )
        okb1 = consts.tile([O, C], F32)
        nc.sync.dma_start(
            out=okb1, in_=bass.AP(tensor=okscr, offset=C, ap=[[0, O], [1, C]])
        )

        # params: f = (ak0-ak1)/(ak1+ak2); ln E = sum(ln u - ln s)
        ak = params.tile([NRC, 288, 3], F32)
        nc.sync.dma_start(out=ak, in_=andk.ap().rearrange("r c i t -> (r c) i t"))
        nc.vector.tensor_scalar(ak, ak, rT, None, AL.mult)
        nc.scalar.activation(out=ak, in_=ak, func=ACT.Exp)
        umat = params.tile([NRC, 288], F32)
        nc.vector.tensor_tensor(umat, ak[:, :, 1], ak[:, :, 2], AL.add)
        smat = params.tile([NRC, 288], F32)
        nc.vector.tensor_tensor(smat, umat, ak[:, :, 0], AL.add)
        rumat = params.tile([NRC, 288], F32)
        nc.vector.reciprocal(out=rumat, in_=umat)
        dmat = params.tile([NRC, 288], F32)
        nc.vector.tensor_tensor(dmat, ak[:, :, 0], ak[:, :, 1], AL.subtract)
        fmat = params.tile([NRC, 288], F32)
        nc.vector.tensor_tensor(fmat, dmat, rumat, AL.mult)
        fbf = params.tile([NRC, 288], BF16)
        nc.vector.tensor_copy(fbf, fmat)
        lnu = params.tile([NRC, 288], F32)
        nc.scalar.activation(out=lnu, in_=umat, func=ACT.Ln)
        lns = params.tile([NRC, 288], F32)
        nc.scalar.activation(out=lns, in_=smat, func=ACT.Ln)
        nc.vector.tensor_tensor(lnu, lnu, lns, AL.subtract)
        esum = params.tile([NRC, 1], F32)
        nc.vector.tensor_reduce(out=esum, in_=lnu, axis=AX.X, op=AL.add)
        nc.scalar.activation(out=esum, in_=esum, func=ACT.Exp)
        nc.sync.dma_start(out=escr.ap().rearrange("o n -> (o n) ()"), in_=esum)

        # stage f: DVE rows (bf16), transposed fp32 (bin block), unary+nullary
        nc.sync.dma_start(
            out=bass.AP(tensor=fbin, offset=0, ap=[[128, DVERC], [1, 128]]),
            in_=fbf[0:DVERC, 160:288],
        )
        nc.sync.dma_start(
            out=bass.AP(tensor=fbinT, offset=0, ap=[[1, NRC], [NRC, 128]]),
            in_=fmat[:, 160:288],
        )
        nc.sync.dma_start(
            out=bass.AP(tensor=fun, offset=0, ap=[[128, NRC], [1, 128]]),
            in_=fbf[:, 32:160],
        )
        nc.sync.dma_start(
            out=bass.AP(tensor=fnul, offset=0, ap=[[P0, NRC], [1, P0]]),
            in_=fbf[:, 0:32],
        )

        # replicated / transposed f tiles
        frep_b = params.tile([128, DVERC, 128], BF16)
        for s in range(4):
            nc.sync.dma_start(
                out=frep_b[s * 32:(s + 1) * 32],
                in_=bass.AP(tensor=fbin, offset=0, ap=[[0, 32], [1, DVERC * 128]]),
            )
        frep_u = params.tile([BPC * O, NRC, 2, P1], BF16)
        for s in range(2):
            nc.sync.dma_start(
                out=frep_u[s * 24:(s + 1) * 24],
                in_=bass.AP(tensor=fun, offset=0, ap=[[0, 24], [1, NRC * 128]]),
            )
        frep_n = params.tile([BPC, NRC, P0], BF16)
        nc.sync.dma_start(
            out=frep_n,
            in_=bass.AP(tensor=fnul, offset=0, ap=[[0, BPC], [1, NRC * P0]]),
        )
        fT = params.tile([128, NRC], F32)
        nc.sync.dma_start(
            out=fT, in_=bass.AP(tensor=fbinT, offset=0, ap=[[NRC, 128], [1, NRC]])
        )

        # ============ P1: unary + nullary products (GPSIMD) ================
        def prod_tree(eng, t_ap, nfeat, out_ap):
            k = nfeat // 2
            while k >= 2:
                eng.tensor_tensor(
                    t_ap[..., 0:k], t_ap[..., 0:k], t_ap[..., k:2 * k], AL.mult
                )
                k //= 2
            eng.tensor_tensor(out_ap, t_ap[..., 0:1], t_ap[..., 1:2], AL.mult)

        mun = mpool.tile([BPC * O, NRC, 2, P1], BF16, tag="mun")
        nc.vector.tensor_tensor(
            mun,
            xu.unsqueeze(1).unsqueeze(1).to_broadcast((BPC * O, NRC, 2, P1)),
            frep_u,
            AL.mult,
        )
        nc.scalar.activation(out=mun, in_=mun, func=ACT.Copy, bias=1.0)
        uprod = small.tile([BPC * O, NRC, 2, 1], F32, tag="uprod")
        prod_tree(nc.vector, mun, P1, uprod)

        mnul = mpool.tile([BPC, NRC, P0], BF16, tag="mnul")
        nc.vector.tensor_tensor(
            mnul, xn.unsqueeze(1).to_broadcast((BPC, NRC, P0)), frep_n, AL.mult
        )
        nc.scalar.activation(out=mnul, in_=mnul, func=ACT.Copy, bias=1.0)
        nprod = small.tile([BPC, NRC, 1], F32, tag="nprod")
        prod_tree(nc.vector, mnul, P0, nprod)

        # N' = nullary-prod * E ; broadcast rows
        erow = small.tile([BPC, NRC], F32, tag="erow")
        nc.sync.dma_start(
            out=erow, in_=bass.AP(tensor=escr, offset=0, ap=[[0, BPC], [1, NRC]])
        )
        n2 = small.tile([BPC, NRC], F32, tag="n2")
        nc.vector.tensor_tensor(n2, nprod[:, :, 0], erow, AL.mult)
        nc.sync.dma_start(out=n2scr.ap(), in_=n2)
        n2rep = small.tile([BPC * O, NRC], F32, tag="n2rep")
        for b in range(BPC):
            nc.sync.dma_start(
                out=n2rep[b * O:(b + 1) * O],
                in_=bass.AP(tensor=n2scr, offset=b * NRC, ap=[[0, O], [1, NRC]]),
            )
        u1n = small.tile([BPC * O, NRC], F32, tag="u1n")
        nc.vector.tensor_tensor(u1n, uprod[:, :, 0, 0], n2rep, AL.mult)

        # log-domain expansion sources (clamped: ln(0) -> NEG_BIG, not -inf)
        logu1 = small.tile([BPC * O, NRC], BF16, tag="logu1")
        nc.scalar.activation(out=logu1, in_=u1n, func=ACT.Ln)
        nc.vector.tensor_scalar(logu1, logu1, NEG_BIG, None, AL.max)
        logu2 = small.tile([BPC * O, NRC], BF16, tag="logu2")
        nc.scalar.activation(out=logu2, in_=uprod[:, :, 1, 0], func=ACT.Ln)
        nc.vector.tensor_scalar(logu2, logu2, NEG_BIG, None, AL.max)

        # ============ P2a: log-path feature sums (ScalarE writes L) ========
        lt = lpool.tile([128, LOGRC, BPC, P], BF16)
        for rc in range(DVERC, NRC):
            nc.scalar.activation(
                out=lt[:, rc - DVERC, :, :],
                in_=xT,
                func=ACT.Ln,
                bias=1.0,
                scale=fT[:, rc:rc + 1],
            )

        # ============ P2b: per-tile pipeline ===============================
        for b in range(BPC):
            gsum = psg.tile([128, NT, NRC], F32, tag="gsum")
            conj = gpool.tile([128, NT, NRC], F32, tag="conj")
            smu = psum.tile([O, C], F32, tag="smu")
            smn = psum.tile([1, C], F32, tag="smn")
            bbts = []
            for t in range(NT):
                rows = TROWS[t]
                # DVE path for rc < DVERC
                m = mpool.tile([128, DVERC, 128], BF16, tag="m")
                nc.vector.tensor_tensor(
                    m[:rows],
                    xball[:rows, b * NT + t, :].unsqueeze(1)
                        .to_broadcast((rows, DVERC, 128)),
                    frep_b[:rows],
                    AL.mult,
                )
                nc.scalar.activation(out=m[:rows], in_=m[:rows], func=ACT.Copy,
                                     bias=1.0)
                bbt = gpool.tile([128, DVERC, 1], F32, tag="bbt")
                k = 64
                while k >= 8:
                    nc.vector.tensor_tensor(m[:rows, :, 0:k], m[:rows, :, 0:k],
                                            m[:rows, :, k:2 * k], AL.mult)
                    k //= 2
                nc.vector.tensor_reduce(out=bbt[:rows], in_=m[:rows, :, 0:8],
                                        axis=AX.X, op=AL.mult)
                bbts.append(bbt)

                # expansions (log domain) + log-path feature sums -> PSUM
                nc.tensor.matmul(gsum[:rows, t, :], exm[:, b, t, 0, :rows],
                                 logu1, start=True, stop=False)
                nc.tensor.matmul(gsum[:rows, t, :], exm[:, b, t, 1, :rows],
                                 logu2, start=False, stop=False)
                for rc in range(DVERC, NRC):
                    nc.tensor.matmul(
                        gsum[:rows, t, rc:rc + 1],
                        lt[:, rc - DVERC, b, t * 128:t * 128 + rows],
                        ones_bf,
                        start=False, stop=(rc == NRC - 1),
                    )

            # conj = exp(gsum); DVE columns then get * bbt
            nc.scalar.activation(out=conj, in_=gsum, func=ACT.Exp)
            for t in range(NT):
                rows = TROWS[t]
                nc.vector.tensor_tensor(
                    conj[:rows, t, 0:DVERC], conj[:rows, t, 0:DVERC],
                    bbts[t][:rows, :, 0], AL.mult,
                )

                # r=2 (binary) out: 1 - prod_c(1 - conj*ok2)
                v = gpool.tile([128, C], F32, tag="v")
                nc.vector.tensor_tensor(v[:rows], conj[:rows, t, 2 * C:3 * C],
                                        okb2[:rows], AL.mult)
                nc.vector.tensor_scalar(v[:rows], v[:rows], -1.0, 1.0, AL.mult, AL.add)
                rm = gpool.tile([128, 1], F32, tag="rm")
                nc.vector.tensor_reduce(out=rm[:rows], in_=v[:rows], axis=AX.X,
                                        op=AL.mult)
                nc.vector.tensor_scalar(rm[:rows], rm[:rows], -1.0, 1.0,
                                        AL.mult, AL.add)
                nc.sync.dma_start(out=out_b.ap()[b:b + 1, t * 128:t * 128 + rows],
                                  in_=rm[:rows])

                # r=0,1: w = ln(1-conj); masked matmul accumulate
                w = gpool.tile([128, 2 * C], F32, tag="w")
                nc.vector.tensor_scalar(w[:rows], conj[:rows, t, 0:2 * C],
                                        -1.0, 1.0, AL.mult, AL.add)
                nc.scalar.activation(out=w[:rows], in_=w[:rows], func=ACT.Ln)
                nc.tensor.matmul(smu, omsk[:rows, t, 0:O], w[:rows, C:2 * C],
                                 start=(t == 0), stop=(t == NT - 1))
                nc.tensor.matmul(smn, omsk[:rows, t, 24:25], w[:rows, 0:C],
                                 start=(t == 0), stop=(t == NT - 1))

            pexp = small.tile([O, C], F32, tag="pexp")
            nc.scalar.activation(out=pexp, in_=smu, func=ACT.Exp)
            pexpn = small.tile([1, C], F32, tag="pexpn")
            nc.scalar.activation(out=pexpn, in_=smn, func=ACT.Exp)

            # nullary out: 1 - prod_c(1 - (1-exp)*ok0)
            t0 = small.tile([1, C], F32, tag="t0")
            nc.vector.tensor_scalar(t0, pexpn, -1.0, 1.0, AL.mult, AL.add)
            nc.vector.tensor_tensor(t0, t0, ok[0:1, 0:C], AL.mult)
            nc.vector.tensor_scalar(t0, t0, -1.0, 1.0, AL.mult, AL.add)
            r0 = small.tile([1, 1], F32, tag="r0")
            nc.vector.tensor_reduce(out=r0, in_=t0, axis=AX.X, op=AL.mult)
            nc.vector.tensor_scalar(r0, r0, -1.0, 1.0, AL.mult, AL.add)
            nc.sync.dma_start(out=out_n.ap()[b:b + 1, :], in_=r0)

            # unary out rows
            t1 = small.tile([O, C], F32, tag="t1")
            nc.vector.tensor_scalar(t1, pexp, -1.0, 1.0, AL.mult, AL.add)
            nc.vector.tensor_tensor(t1, t1, okb1, AL.mult)
            nc.vector.tensor_scalar(t1, t1, -1.0, 1.0, AL.mult, AL.add)
            r1 = small.tile([O, 1], F32, tag="r1")
            nc.vector.tensor_reduce(out=r1, in_=t1, axis=AX.X, op=AL.mult)
            nc.vector.tensor_scalar(r1, r1, -1.0, 1.0, AL.mult, AL.add)
            nc.sync.dma_start(out=out_u.ap()[b:b + 1, :], in_=r1)

    nc.compile()
    return nc


_MODULE = None
LAST_RESULT = None


def _install_ntff_shim():
    """Provide antenv.axon_hooks (NTFF profile hook) if the image lacks it.
    Only matters when BASS_TRACE=1; no-op otherwise."""
    import sys
    import types
    try:
        import antenv.axon_hooks  # noqa: F401
        return
    except ImportError:
        pass
    hook = None
    try:
        from trn_agent_boot.trn_boot import _ntff_profile_via_ctypes
        hook = _ntff_profile_via_ctypes("/opt/axon/libaxon_pjrt.so")
    except Exception:
        hook = None
    mod = types.ModuleType("antenv.axon_hooks")
    state = {"hook": hook}
    mod.get_axon_ntff_profile_hook = lambda: state["hook"]
    mod.set_axon_ntff_profile_hook = lambda h: state.update(hook=h)
    sys.modules["antenv.axon_hooks"] = mod
    try:
        import antenv
        antenv.axon_hooks = mod
    except Exception:
        pass


def kernel(nullary, unary, binary, and_kernel, or_kernel, temperature):
    global _MODULE, LAST_RESULT
    nullary = np.asarray(nullary, dtype=np.float32)
    unary = np.asarray(unary, dtype=np.float32)
    binary = np.asarray(binary, dtype=np.float32)
    and_kernel = np.ascontiguousarray(np.asarray(and_kernel, dtype=np.float32))
    or_kernel = np.ascontiguousarray(np.asarray(or_kernel, dtype=np.float32))
    temperature = np.asarray(temperature, dtype=np.float32)

    # host prep: per-permutation pair rows (+ transposed copy), shard by batch
    binf = binary.reshape(B, P, P2)                       # p = a*23 + idx(bo|a)
    xbin_full = np.concatenate([binf, binf[:, _SWAP, :]], axis=-1).astype(_BF)
    xbinT_full = np.ascontiguousarray(xbin_full.transpose(0, 2, 1))
    xun_full = unary.astype(_BF)
    xnul_full = nullary.astype(_BF)

    if _MODULE is None:
        _MODULE = _build_module()
    nc = _MODULE

    in_maps = []
    for c in range(NCORES):
        b0 = c * BPC
        in_maps.append({
            "xbin": np.ascontiguousarray(xbin_full[b0:b0 + BPC]),
            "xbinT": np.ascontiguousarray(xbinT_full[b0:b0 + BPC]),
            "xun": np.ascontiguousarray(xun_full[b0:b0 + BPC].reshape(BPC * O, P1)),
            "xnul": np.ascontiguousarray(xnul_full[b0:b0 + BPC]),
            "andk": and_kernel,
            "ork": or_kernel,
            "tempr": temperature.reshape(1, 1).astype(np.float32),
            "ormasks": _ORMASKS,
            "exmasks": _EXMASKS,
        })

    _install_ntff_shim()
    LAST_RESULT = run_bass_kernel_spmd(nc, in_maps, core_ids=list(range(NCORES)))
    res = LAST_RESULT.results

    nullary_out = np.concatenate([r["out_n"] for r in res], axis=0).reshape(B, 1)
    unary_out = np.concatenate([r["out_u"] for r in res], axis=0).reshape(B, O, 1)
    binary_out = np.concatenate([r["out_b"] for r in res], axis=0).reshape(B, O, O - 1, 1)
    return nullary_out, unary_out, binary_out
